# revision 76
# baseline (speedup 1.0000x reference)
"""Trainium2 Bass kernel for nn_MultiHeadAttention_88536455840315.

Math notes (vs the jax reference):
  - The second einsum (log_probs[..., None] * attn) @ v factors to
    log_probs[..., None] * (attn @ v) because log_probs does not depend on
    the key index.  So only two big attention matmuls are needed.
  - Softmax is computed without max subtraction: dots ~ N(0,1) here, so
    exp(dots*scale) never overflows fp32.
  - sumexp is fused into the attn@v matmul as a ones column appended to V.

Sharding (8 cores): core c handles batch c//4 and query rows
(c%4)*512 .. +512 of that batch.  Each core computes the full K/V for its
batch (replicated within the 4-core group; cross-core collectives are
either unsimulable -- remote_dma sem waits deadlock the single-core
timeline sim -- or cost-prohibitive via collective_compute: 15us constant
overhead + 40GB/s).  The per-core query offset is realized by rolling the
batch rows host-side so that each core's queries are always rows 0:512
(softmax is permutation-invariant over keys, so rolling K/V order is
exact).

Performance structure (vs the 186us previous version):
  - x and w_qkv are host-cast to bf16; x^T is produced by DMA xbar
    transposes (InstDmaTransposeAnt, 14ns per 16x128 tile) straight from
    HBM into SBUF.  This deletes all 96 PE transposes, their PSUM->SBUF
    drains, and the separate x load; the whole phase-1 x pipeline is gone
    and the first dots start ~8us in instead of ~16us.
  - With the transposes gone, the V projection moved into the head loop
    as PE filler.  PE is the pacing engine (~140us busy: QKV projections
    1.0 cycles/col in bf16, dots fp32r, attn@V bf16); ACT runs ~100us of
    softmax exp underneath it.  fp8/DoubleRow was measured (numpy) to
    push rel-err to 2.6e-2..6e-2 -- the cross-head variance statistics
    amplify product errors ~10x -- so everything stays bf16/fp32r.
  - attn@V runs in the [query, dim] orientation: exp tiles (bf16) are
    the PE stationary operand and V+ones (bf16) is the short (65-col)
    moving operand; the product lands directly in [q, d] layout.
  - attn_v lags dots/exp via a 30-buffer bf16 exp ring, and all PE
    filler (attn_v of earlier heads, K^T/Q^T chunk projections, V
    projection pieces) is emitted in small units INTERLEAVED between the
    dots tiles of each head: the dps double-buffer would otherwise stall
    the in-order PE queue behind a waiting dots matmul.
  - Tail: head 11 + normalize + mean/var + log-prob chain run PER QUERY
    TILE, pipelined across DVE (chains), ACT (squares, Ln, copies) and
    Pool, so the OH^T transposes + output projection (one continuous PE
    burst, bf16) start ~5us after the last exp instead of ~12us.  OH and
    W_out are bf16 (halves W_out DMA, 1.0 cyc/col transposes).
  - mean/var come from running partial sums (ACCS/ACCQ on Pool, head 11
    folded in at the tail), and the bias is folded into the output
    matmul as a rank-1 ones x bias product so y streams from fin tiles.
"""

import sys

if "/opt/trn_rl_repo" not in sys.path:
    sys.path.insert(0, "/opt/trn_rl_repo")

import ml_dtypes
import numpy as np

import concourse.bass as bass
import concourse.mybir as mybir
import concourse.tile as tile
from concourse import bacc
from concourse import bass_utils
from concourse.masks import make_identity

F32 = mybir.dt.float32
F32R = mybir.dt.float32r
BF16 = mybir.dt.bfloat16
AF = mybir.ActivationFunctionType
ALU = mybir.AluOpType
AX = mybir.AxisListType

B, N, E = 2, 2048, 768
H, DH = 12, 64
HD = H * DH            # 768
NQ = 512               # query rows per core
SCALE = DH ** -0.5
LOG2PI = float(np.log(2.0 * np.pi))
CONST = -0.5 * DH * LOG2PI   # -32*log(2*pi)

NE = E // 128          # 6 chunks of the embedding dim
NN = N // 128          # 16 chunks of the sequence
NQT = NQ // 128        # 4 query tiles
D1 = DH + 1            # head dim + sumexp column


def _emit(tc):
    nc = tc.nc
    xb = nc.dram_tensor("xb", [N, E], BF16, kind="ExternalInput")
    xb_ap = xb.ap()
    wqkv = nc.dram_tensor("wqkv", [E, 3 * HD], BF16, kind="ExternalInput")
    wqkv_ap = wqkv.ap()
    wout = nc.dram_tensor("wout", [HD, E], BF16, kind="ExternalInput").ap()
    bout_t = nc.dram_tensor("bout", [E], F32R, kind="ExternalInput")
    y = nc.dram_tensor("y", [NQ, E], F32, kind="ExternalOutput").ap()

    with tc.tile_pool(name="consts", bufs=1) as consts, \
         tc.tile_pool(name="big", bufs=1) as big, \
         tc.tile_pool(name="wop", bufs=1) as wop:
        # Load the one activation table that covers every function this
        # kernel uses (exp, ln, square, copy = set 6,
        # "natural_log_exp_and_others" in act_info.json).  Without this the
        # table-load pass picks "exp_and_others" first and a second 1283ns
        # load for Ln lands mid-chain in the latency-critical tail.
        nc.scalar.add_instruction(mybir.InstLoadActFuncSet(
            name=nc.scalar.bass.get_next_instruction_name(),
            act_func_set_id=6, ins=[], outs=[]))
        ident = consts.tile([128, 128], F32, name="ident", tag="ident")
        make_identity(nc, ident)
        identb = consts.tile([128, 128], BF16, name="identb", tag="identb")
        nc.vector.tensor_copy(identb, ident)
        ones_f = consts.tile([1, 128], F32, name="onesf", tag="onesf")
        nc.gpsimd.memset(ones_f, 1.0)
        ones_r = consts.tile([1, 128], F32R, name="ones", tag="ones")
        nc.vector.tensor_copy(ones_r, ones_f)

        # persistent SBUF tensors
        XT = [big.tile([128, N], BF16, name=f"xt{i}", tag=f"xt{i}")
              for i in range(NE)]
        VA = [big.tile([128, H, D1], BF16, name=f"va{j}", tag=f"va{j}")
              for j in range(NN)]
        PROD = big.tile([128, NQT, H, DH], BF16, name="prod", tag="prod")
        ACCS = big.tile([128, NQT, DH], F32, name="accs", tag="accs")
        ACCQ = big.tile([128, NQT, DH], F32, name="accq", tag="accq")
        MA = big.tile([128, NQT, DH], F32, name="ma", tag="ma")
        QA = big.tile([128, NQT, DH], F32, name="qa", tag="qa")
        OHQ = big.tile([128, NQT, H * DH], BF16, name="ohq", tag="ohq")
        bias = big.tile([1, E], F32R, name="bias", tag="bias")

        # ones column for the fused sumexp
        for va in VA:
            nc.gpsimd.memset(va[:, :, DH:D1], 1.0)

        with tc.tile_pool(name="jps", bufs=2, space="PSUM") as jps, \
             tc.tile_pool(name="wvp", bufs=1) as wvp, \
             tc.tile_pool(name="wqk", bufs=2) as wqk, \
             tc.tile_pool(name="ktp", bufs=2) as ktp, \
             tc.tile_pool(name="qtp", bufs=2) as qtp:

            # ---------------- K^T / Q^T projection helpers ----------------
            KT = {}
            QT = {}

            def load_wqk_pair(p):
                """W_q and W_k column chunks 2p,2p+1 as [128, 6, 256]
                tiles: paired loads give 512-byte DMA descriptors (half
                the per-descriptor latency of 256-byte ones) and half the
                HWDGE calls."""
                tq = wqk.tile([128, NE, 256], BF16, name="wq6", tag="wq6")
                tk = wqk.tile([128, NE, 256], BF16, name="wk6", tag="wk6")
                for t, col0 in ((tq, p * 256), (tk, HD + p * 256)):
                    nc.sync.dma_start(out=t, in_=bass.AP(
                        tensor=wqkv, offset=col0,
                        ap=[[3 * HD, 128], [128 * 3 * HD, NE], [1, 256]]))
                return tq, tk

            WVA = wvp.tile([128, NE, 512], BF16, name="wva", tag="wva")
            WVB = wvp.tile([128, NE, 256], BF16, name="wvb", tag="wvb")

            # -------- prologue DMA schedule (SP hwdge, priority order) ----
            # HWDGE issue is serialized at ~625ns/call, so DMAs are
            # consolidated (one strided call per W tensor) and ordered by
            # when the PE needs them: wqk0 + the query/first-key x^T spans
            # gate the first dots; later key spans gate the K^T pieces and
            # the V projection fillers.
            WQKP = {0: load_wqk_pair(0)}
            for r0 in (0, 512, 1024, 1536):
                for e in range(NE):
                    nc.sync.dma_start_transpose(
                        out=XT[e][:, r0:r0 + 512],
                        in_=xb_ap[r0:r0 + 512, e * 128:(e + 1) * 128])
            nc.sync.dma_start(out=WVA, in_=bass.AP(
                tensor=wqkv, offset=2 * HD,
                ap=[[3 * HD, 128], [128 * 3 * HD, NE], [1, 512]]))
            nc.sync.dma_start(out=WVB, in_=bass.AP(
                tensor=wqkv, offset=2 * HD + 512,
                ap=[[3 * HD, 128], [128 * 3 * HD, NE], [1, 256]]))

            # ---------------- V projection (filler units) -----------------
            def v_proj_a(nb):
                """V heads 0-7 for key block nb."""
                psA = jps.tile([128, 512], F32, name="vpa", tag="jp")
                for e in range(NE):
                    nc.tensor.matmul(
                        psA, XT[e][:, nb * 128:(nb + 1) * 128],
                        WVA[:, e, :],
                        start=(e == 0), stop=(e == NE - 1))
                nc.vector.tensor_copy(
                    VA[nb][:, 0:8, 0:DH],
                    psA.rearrange("p (h d) -> p h d", h=8))

            def v_proj_b(nb):
                """V heads 8-11 for key block nb."""
                psB = jps.tile([128, 512], F32, name="vpb", tag="jp")
                for e in range(NE):
                    nc.tensor.matmul(
                        psB[:, 0:256], XT[e][:, nb * 128:(nb + 1) * 128],
                        WVB[:, e, :],
                        start=(e == 0), stop=(e == NE - 1))
                nc.vector.tensor_copy(
                    VA[nb][:, 8:12, 0:DH],
                    psB[:, 0:256].rearrange("p (h d) -> p h d", h=4))

            def qk_proj_units(kc, tq, tk):
                """Closures: Q^T piece then 4 K^T pieces for chunk kc.
                tq/tk hold a kc-PAIR; co selects this kc's 128 columns."""
                co = (kc % 2) * 128
                kt = ktp.tile([128, N], F32R, name="kt", tag="kt")
                qt = qtp.tile([128, NQ], F32R, name="qt", tag="qt")
                KT[kc] = kt
                QT[kc] = qt

                def q_piece():
                    ps = jps.tile([128, 512], F32, name="qp", tag="jp")
                    for e in range(NE):
                        nc.tensor.matmul(ps, tq[:, e, co:co + 128],
                                         XT[e][:, 0:NQ],
                                         start=(e == 0), stop=(e == NE - 1))
                    nc.vector.tensor_copy(qt, ps)

                def k_piece(nb):
                    ps = jps.tile([128, 512], F32, name="kp", tag="jp")
                    for e in range(NE):
                        nc.tensor.matmul(
                            ps, tk[:, e, co:co + 128],
                            XT[e][:, nb * 512:(nb + 1) * 512],
                            start=(e == 0), stop=(e == NE - 1))
                    nc.vector.tensor_copy(kt[:, nb * 512:(nb + 1) * 512],
                                          ps)
                return [q_piece] + [
                    (lambda nb=nb: k_piece(nb)) for nb in range(4)]

            WO = []

            # ---------------- attention main loop -------------------------
            with tc.tile_pool(name="dps", bufs=2, space="PSUM") as dps, \
                 tc.tile_pool(name="pps", bufs=2, space="PSUM") as pps, \
                 tc.tile_pool(name="expp", bufs=30) as expp:

                # prologue PE: Q/K(0) projections gate the first dots
                u0 = qk_proj_units(0, *WQKP[0])
                u0[0]()
                u0[1]()
                PRE0 = {2: u0[2], 4: u0[3], 6: u0[4]}

                EXP = {}   # h -> list of 8 exp tiles
                PPS = {}   # h -> product psum tile

                def dots_exp(h, fillers=(), pre=None):
                    """dots+exp for head h with PE filler units emitted
                    between dots tiles (the dps double-buffer makes dots
                    exp-paced; interleaved filler keeps PE busy).  `pre`
                    maps tile index -> unit emitted before that tile (for
                    just-in-time K^T pieces of head 0)."""
                    fillers = list(fillers)
                    kc, pofs = h // 2, (h % 2) * DH
                    kt, qt = KT[kc], QT[kc]
                    qth = qt[pofs:pofs + DH, :]
                    exs = []
                    for jj in range(8):
                        if pre and jj in pre:
                            pre.pop(jj)()
                        dt_ = dps.tile([128, 2, NQ], F32, name="dots",
                                       tag="dots")
                        for k in range(2):
                            jb = jj * 2 + k
                            nc.tensor.matmul(
                                dt_[:, k, :],
                                kt[pofs:pofs + DH, jb * 128:(jb + 1) * 128],
                                qth, start=True, stop=True)
                        ex = expp.tile([128, 2, NQ], BF16, name="expd",
                                       tag="expd")
                        nc.scalar.activation(out=ex, in_=dt_, func=AF.Exp,
                                             scale=SCALE)
                        exs.append(ex)
                        if jj >= 1 and fillers:
                            fillers.pop(0)()
                    EXP[h] = exs
                    for f in fillers:
                        f()

                def attn_v_qt(h, qt_i):
                    """attn@V for head h, one query tile."""
                    exs = EXP[h]
                    if qt_i == 0:
                        PPS[h] = pps.tile([128, NQT, D1], F32, name="pp",
                                          tag="pp")
                    pp = PPS[h]
                    for jb in range(NN):
                        ex = exs[jb // 2]
                        st = ex[:, jb % 2, qt_i * 128:(qt_i + 1) * 128]
                        nc.tensor.matmul(
                            pp[:, qt_i, :], st, VA[jb][:, h, :],
                            start=(jb == 0), stop=(jb == NN - 1))
                    if qt_i == NQT - 1:
                        EXP.pop(h)

                def normalize(h):
                    """h <= 10: product write + running sums on Pool."""
                    acc_eng = nc.gpsimd
                    pp = PPS.pop(h)
                    rsh = big.tile([128, NQT], F32, name="rsh", tag="rsh",
                                   bufs=3)
                    nc.vector.reciprocal(rsh, pp[:, :, DH:D1])
                    pvh = bass.AP(tensor=PROD.tensor,
                                  offset=PROD.offset + h * DH,
                                  ap=[PROD.ap[0], [H * DH, NQT], [1, DH]])
                    rsh_bc = bass.AP(tensor=rsh.tensor, offset=rsh.offset,
                                     ap=[rsh.ap[0], [1, NQT], [0, DH]])
                    nc.vector.tensor_tensor(out=pvh, in0=pp[:, :, 0:DH],
                                            in1=rsh_bc, op=ALU.mult)
                    if h == 0:
                        acc_eng.tensor_copy(ACCS, pvh)
                        acc_eng.tensor_tensor(out=ACCQ, in0=pvh, in1=pvh,
                                              op=ALU.mult)
                    else:
                        sqh = big.tile([128, NQT, DH], F32, name="sqh",
                                       tag="sqh", bufs=2)
                        acc_eng.tensor_tensor(out=sqh, in0=pvh, in1=pvh,
                                              op=ALU.mult)
                        acc_eng.tensor_tensor(out=ACCS, in0=ACCS, in1=pvh,
                                              op=ALU.add)
                        acc_eng.tensor_tensor(out=ACCQ, in0=ACCQ, in1=sqh,
                                              op=ALU.add)

                done_av = 0

                def av_units(h):
                    """attn_v for head h as 4 filler units; normalize
                    rides with the last qt."""
                    us = [(lambda q=q: attn_v_qt(h, q))
                          for q in range(NQT - 1)]

                    def last():
                        attn_v_qt(h, NQT - 1)
                        normalize(h)
                    return us + [last]

                def drain_units(upto):
                    nonlocal done_av
                    us = []
                    while done_av < upto:
                        us += av_units(done_av)
                        done_av += 1
                    return us

                for h in range(H):
                    fillers = []
                    if h % 2 == 1 and h < H - 1:
                        if h in (1, 5):
                            # prefetch the next W_qk pair well ahead
                            WQKP[(h + 3) // 4] = load_wqk_pair((h + 3) // 4)
                        kc = h // 2 + 1
                        fillers += qk_proj_units(kc, *WQKP[kc // 2])
                    if h == 1:
                        fillers += [(lambda nb=nb: v_proj_a(nb))
                                    for nb in range(0, 8)]
                    if h == 2:
                        fillers = [(lambda nb=nb: v_proj_a(nb))
                                   for nb in range(8, 16)] \
                            + drain_units(2) + fillers
                    elif h >= 3:
                        upto = {8: 7, 9: 8, 10: 9, 11: 11}.get(h, h)
                        fillers = drain_units(upto) + fillers
                    if 3 <= h <= 7:
                        fillers += [(lambda k=k: v_proj_b(2 * (h - 3) + k))
                                    for k in range(2)]
                    elif h == 8:
                        fillers += [(lambda k=k: v_proj_b(k))
                                    for k in range(10, 13)]
                    elif h == 10:
                        fillers = [(lambda k=k: v_proj_b(k))
                                   for k in range(13, 16)] + fillers
                    if h == 8:
                        # W_out + bias loads, overlapped with late attention
                        nc.sync.dma_start(out=bias, in_=bass.AP(
                            tensor=bout_t, offset=0, ap=[[0, 1], [1, E]]))
                        wo_t = wop.tile([128, NE, E], BF16, name="wo",
                                        tag="wo")
                        nc.sync.dma_start(out=wo_t, in_=bass.AP(
                            tensor=wout.tensor, offset=0,
                            ap=[[E, 128], [128 * E, NE], [1, E]]))
                        WO.append(wo_t)
                    if h == H - 1:
                        def ma_qa():
                            nc.vector.tensor_scalar_mul(MA, ACCS, 1.0 / H)
                            nc.vector.tensor_scalar_mul(QA, ACCQ,
                                                        1.0 / (H - 1))
                        fillers.append(ma_qa)
                    dots_exp(h, fillers, pre=PRE0 if h == 0 else None)
                for u in drain_units(H - 1):
                    u()



                # ------- tail: head 11 + statistics -----------------------
                # attn_v(11) lands on PE back-to-back (a per-qt interleave
                # would WAR-serialize each start=True against normalize
                # reads through the PSUM zero region); then per-qt
                # normalize/mean/var/log-prob chains run on DVE (qt 0/2)
                # and Pool (qt 1/3) with squares + Ln on ACT, so the
                # output projection (emitted after this scope) starts on
                # qt 0 while later qts still compute.  Scratch transposes
                # reading the early stats tiles keep the tensor engine's
                # p-state ramp alive through the otherwise-idle window.
                for qt_i in range(NQT):
                    attn_v_qt(11, qt_i)
                warm = jps.tile([128, 128], F32, name="warm", tag="jp")

                def keep_warm(src, n):
                    out = warm[0:64, :]
                    for _ in range(n):
                        nc.tensor.transpose(out, src, ident)

                with tc.tile_pool(name="wkp", bufs=1) as wkp:
                    mean = wkp.tile([128, NQT, DH], F32, name="mean",
                                    tag="mean")
                    rvar = wkp.tile([128, NQT, DH], F32, name="rvar",
                                    tag="rvar")
                    cs = wkp.tile([128, NQT], F32, name="cs", tag="cs")
                    pp = PPS.pop(11)
                    for qt_i in range(NQT):
                        rshq = wkp.tile([128, 1], F32, name="rshq",
                                        tag="rshq", bufs=4)
                        nc.vector.reciprocal(rshq, pp[:, qt_i, DH:D1])
                        p11q = bass.AP(
                            tensor=PROD.tensor,
                            offset=PROD.offset + qt_i * H * DH + 11 * DH,
                            ap=[PROD.ap[0], [1, DH]])
                        rsh_bc = bass.AP(tensor=rshq.tensor,
                                         offset=rshq.offset,
                                         ap=[rshq.ap[0], [0, DH]])
                        nc.vector.tensor_tensor(out=p11q,
                                                in0=pp[:, qt_i, 0:DH],
                                                in1=rsh_bc, op=ALU.mult)
                        mq = mean[:, qt_i]
                        nc.vector.scalar_tensor_tensor(
                            out=mq, in0=p11q, scalar=1.0 / H,
                            in1=MA[:, qt_i], op0=ALU.mult, op1=ALU.add)
                        sqq = wkp.tile([128, DH], F32, name="sqq",
                                       tag="sqq", bufs=2)
                        nc.scalar.activation(out=sqq, in_=p11q,
                                             func=AF.Square)
                        qvq = wkp.tile([128, DH], F32, name="qvq",
                                       tag="qvq", bufs=2)
                        nc.vector.scalar_tensor_tensor(
                            out=qvq, in0=sqq, scalar=1.0 / (H - 1),
                            in1=QA[:, qt_i], op0=ALU.mult, op1=ALU.add)
                        m2q = wkp.tile([128, DH], F32, name="m2q",
                                       tag="m2q", bufs=2)
                        nc.vector.scalar_tensor_tensor(
                            out=m2q, in0=mq, scalar=H / (H - 1.0),
                            in1=mq, op0=ALU.mult, op1=ALU.mult)
                        varq = wkp.tile([128, DH], F32, name="varq",
                                        tag="varq", bufs=2)
                        nc.vector.tensor_tensor(out=varq, in0=qvq,
                                                in1=m2q, op=ALU.subtract)
                        rvq = rvar[:, qt_i]
                        nc.vector.reciprocal(rvq, varq)
                        lvq = wkp.tile([128, DH], F32, name="lvq",
                                       tag="lvq", bufs=2)
                        nc.scalar.activation(out=lvq, in_=varq, func=AF.Ln)
                        sq_ = wkp.tile([128, 1], F32, name="Sq", tag="Sq",
                                       bufs=2)
                        nc.vector.reduce_sum(sq_, lvq, axis=AX.X)
                        nc.vector.tensor_scalar(
                            out=cs[:, qt_i:qt_i + 1], in0=sq_, scalar1=-1.0,
                            scalar2=CONST, op0=ALU.mult, op1=ALU.add)
                        # log-prob chain over all heads for this qt
                        pvq = bass.AP(tensor=PROD.tensor,
                                      offset=PROD.offset + qt_i * H * DH,
                                      ap=[PROD.ap[0], [DH, H], [1, DH]])
                        diff = wkp.tile([128, H, DH], BF16, name="diff",
                                        tag="diff", bufs=2)
                        mean_bc = bass.AP(
                            tensor=mean.tensor,
                            offset=mean.offset + qt_i * DH,
                            ap=[mean.ap[0], [0, H], [1, DH]])
                        rvar_bc = bass.AP(
                            tensor=rvar.tensor,
                            offset=rvar.offset + qt_i * DH,
                            ap=[rvar.ap[0], [0, H], [1, DH]])
                        lp0 = wkp.tile([128, H], F32, name="lp0",
                                       tag="lp0", bufs=2)
                        lp = wkp.tile([128, H], F32, name="lp", tag="lp",
                                      bufs=2)
                        # qt 0 (which gates the whole output projection)
                        # splits heads 0-7 (DVE + ACT square) / 8-11 (Pool,
                        # in-engine square) so its first OH piece lands
                        # ~1.5us earlier; qt 2 runs on Pool, the rest DVE
                        if qt_i in (0, 3):
                            # a-half square in-engine: the ACT queue would
                            # serialize it ahead of Ln and delay cs
                            hsplits = [(0, 8, nc.vector, qt_i == 3),
                                       (8, H, nc.gpsimd, False)]
                        elif qt_i == 1:
                            hsplits = [(0, H, nc.gpsimd, True)]
                        else:
                            hsplits = [(0, H, nc.vector, True)]
                        for h0, h1, eng, act_sq in hsplits:
                            nh = h1 - h0
                            dfs = diff[:, h0:h1, :]
                            sub_bc = bass.AP(
                                tensor=mean.tensor,
                                offset=mean.offset + qt_i * DH,
                                ap=[mean.ap[0], [0, nh], [1, DH]])
                            pvs = bass.AP(
                                tensor=PROD.tensor,
                                offset=PROD.offset + qt_i * H * DH
                                + h0 * DH,
                                ap=[PROD.ap[0], [DH, nh], [1, DH]])
                            eng.tensor_tensor(out=dfs, in0=pvs,
                                              in1=sub_bc, op=ALU.subtract)
                            if act_sq:
                                nc.scalar.activation(out=dfs, in_=dfs,
                                                     func=AF.Square)
                            else:
                                eng.tensor_tensor(out=dfs, in0=dfs,
                                                  in1=dfs, op=ALU.mult)
                            rv_bc = bass.AP(
                                tensor=rvar.tensor,
                                offset=rvar.offset + qt_i * DH,
                                ap=[rvar.ap[0], [0, nh], [1, DH]])
                            eng.tensor_tensor(out=dfs, in0=dfs, in1=rv_bc,
                                              op=ALU.mult)
                            nc.vector.reduce_sum(lp0[:, h0:h1], dfs,
                                                 axis=AX.X)
                            nc.vector.tensor_scalar(
                                out=lp[:, h0:h1], in0=lp0[:, h0:h1],
                                scalar1=0.25,
                                scalar2=cs[:, qt_i:qt_i + 1],
                                op0=ALU.mult, op1=ALU.add)
                        # OH written in two pieces (heads 0-7, then 8-11)
                        # so the first four OH^T transposes (columns 0:512)
                        # start before the full tile is done
                        ohv_a = bass.AP(tensor=OHQ.tensor,
                                        offset=OHQ.offset + qt_i * H * DH,
                                        ap=[OHQ.ap[0], [DH, 8], [1, DH]])
                        ohv_b = bass.AP(
                            tensor=OHQ.tensor,
                            offset=OHQ.offset + qt_i * H * DH + 8 * DH,
                            ap=[OHQ.ap[0], [DH, 4], [1, DH]])
                        pvq_a = bass.AP(tensor=PROD.tensor,
                                        offset=PROD.offset + qt_i * H * DH,
                                        ap=[PROD.ap[0], [DH, 8], [1, DH]])
                        pvq_b = bass.AP(
                            tensor=PROD.tensor,
                            offset=PROD.offset + qt_i * H * DH + 8 * DH,
                            ap=[PROD.ap[0], [DH, 4], [1, DH]])
                        lp_a = bass.AP(tensor=lp.tensor, offset=lp.offset,
                                       ap=[lp.ap[0], [1, 8], [0, DH]])
                        lp_b = bass.AP(tensor=lp.tensor,
                                       offset=lp.offset + 8,
                                       ap=[lp.ap[0], [1, 4], [0, DH]])
                        eng_a = nc.gpsimd if qt_i == 1 else nc.vector
                        eng_b = nc.gpsimd if qt_i != 2 else nc.vector
                        eng_a.tensor_tensor(out=ohv_a, in0=pvq_a,
                                            in1=lp_a, op=ALU.mult)
                        eng_b.tensor_tensor(out=ohv_b, in0=pvq_b,
                                            in1=lp_b, op=ALU.mult)

        # ---------------- OH^T + output projection ------------------------
        # (attention PSUM pools are closed; tp2/fps reuse those banks)
        with tc.tile_pool(name="tp2", bufs=2, space="PSUM") as tp2p, \
             tc.tile_pool(name="fps", bufs=3, space="PSUM") as fps, \
             tc.tile_pool(name="ohtp", bufs=1) as ohtp, \
             tc.tile_pool(name="finp", bufs=2) as finp:
            OHT = ohtp.tile([128, NE, NQ], BF16, name="oht", tag="oht")
            for qt_i in range(NQT):
                oh_q = OHQ[:, qt_i]
                qs = slice(qt_i * 128, (qt_i + 1) * 128)
                # transposes collect in two PSUM tiles so the SBUF drain is
                # 2 batched copies instead of 6 (ACT access overhead)
                tp4 = tp2p.tile([128, 4, 128], BF16, name="t4", tag="t4")
                tp2 = tp2p.tile([128, 2, 128], BF16, name="t2", tag="t2")
                # chunks 0-3 (heads 0-7) transpose, drain, and start the
                # psA accumulation before the heads-8-11 OH piece is ready
                for c in range(4):
                    nc.tensor.transpose(
                        tp4[:, c, :], oh_q[:, c * 128:(c + 1) * 128],
                        identb)
                nc.scalar.copy(OHT[:, 0:4, qs], tp4)
                psA = fps.tile([128, 512], F32, name="fA", tag="f")
                psB = fps.tile([128, 256], F32, name="fB", tag="f")
                nc.tensor.matmul(psA, ones_r, bias[:, 0:512],
                                 start=True, stop=False)
                for c in range(4):
                    nc.tensor.matmul(psA, OHT[:, c, qs],
                                     WO[0][:, c, 0:512],
                                     start=False, stop=False)
                for c in (4, 5):
                    nc.tensor.transpose(
                        tp2[:, c - 4, :], oh_q[:, c * 128:(c + 1) * 128],
                        identb)
                nc.scalar.copy(OHT[:, 4:6, qs], tp2)
                for c in (4, 5):
                    nc.tensor.matmul(psA, OHT[:, c, qs],
                                     WO[0][:, c, 0:512],
                                     start=False, stop=(c == NE - 1))
                nc.tensor.matmul(psB, ones_r, bias[:, 512:768],
                                 start=True, stop=False)
                for c in range(NE):
                    nc.tensor.matmul(psB, OHT[:, c, qs],
                                     WO[0][:, c, 512:768],
                                     start=False, stop=(c == NE - 1))
                fin = finp.tile([128, E], F32, name="fin", tag="fin")
                nc.scalar.copy(fin[:, 0:512], psA)
                nc.sync.dma_start(out=y[qt_i * 128:(qt_i + 1) * 128, 0:512],
                                  in_=fin[:, 0:512])
                if qt_i == NQT - 1:
                    nc.vector.tensor_copy(fin[:, 512:768], psB)
                else:
                    nc.scalar.copy(fin[:, 512:768], psB)
                nc.sync.dma_start(out=y[qt_i * 128:(qt_i + 1) * 128,
                                        512:768],
                                  in_=fin[:, 512:768])


_NC_CACHE = {}


def _get_nc():
    if "nc" not in _NC_CACHE:
        nc = bacc.Bacc("TRN2", target_bir_lowering=False, debug=False,
                       num_devices=8)
        with tile.TileContext(nc) as tc:
            _emit(tc)
        nc.compile()
        _NC_CACHE["nc"] = nc
    return _NC_CACHE["nc"]


def kernel(x, w_qkv, w_out, b_out):
    x = np.ascontiguousarray(x, dtype=np.float32)
    w_qkv = np.ascontiguousarray(w_qkv, dtype=np.float32)
    w_out = np.ascontiguousarray(w_out, dtype=np.float32)
    b_out = np.ascontiguousarray(b_out, dtype=np.float32)
    assert x.shape == (B, N, E)

    nc = _get_nc()
    xb16 = x.astype(ml_dtypes.bfloat16)
    wq16 = np.ascontiguousarray(w_qkv.astype(ml_dtypes.bfloat16))
    wo16 = np.ascontiguousarray(w_out.astype(ml_dtypes.bfloat16))
    in_maps = []
    for c in range(8):
        beta, qoff = c // 4, (c % 4) * NQ
        xbc = np.ascontiguousarray(np.roll(xb16[beta], -qoff, axis=0))
        in_maps.append({"xb": xbc, "wqkv": wq16, "wout": wo16,
                        "bout": b_out})
    res = bass_utils.run_bass_kernel_spmd(nc, in_maps, core_ids=list(range(8)))
    out = np.empty((B, N, E), dtype=np.float32)
    for c in range(8):
        beta, qoff = c // 4, (c % 4) * NQ
        out[beta, qoff:qoff + NQ, :] = res.results[c]["y"]
    return out


# revision 77
# speedup vs baseline: 1.0544x; 1.0544x over previous
"""Trainium2 Bass kernel for nn_MultiHeadAttention_88536455840315.

Math notes (vs the jax reference):
  - The second einsum (log_probs[..., None] * attn) @ v factors to
    log_probs[..., None] * (attn @ v) because log_probs does not depend on
    the key index.  So only two big attention matmuls are needed.
  - Softmax is computed without max subtraction: dots ~ N(0,1) here, so
    exp(dots*scale) never overflows fp32.
  - sumexp is fused into the attn@v matmul as a ones column appended to V.

Sharding (8 cores): core c handles batch c//4 and query rows
(c%4)*512 .. +512 of that batch.  Each core computes the full K/V for its
batch (replicated within the 4-core group; cross-core collectives are
either unsimulable -- remote_dma sem waits deadlock the single-core
timeline sim -- or cost-prohibitive via collective_compute: 15us constant
overhead + 40GB/s).  The per-core query offset is realized by rolling the
batch rows host-side so that each core's queries are always rows 0:512
(softmax is permutation-invariant over keys, so rolling K/V order is
exact).

Performance structure (vs the 186us previous version):
  - x and w_qkv are host-cast to bf16; x^T is produced by DMA xbar
    transposes (InstDmaTransposeAnt, 14ns per 16x128 tile) straight from
    HBM into SBUF.  This deletes all 96 PE transposes, their PSUM->SBUF
    drains, and the separate x load; the whole phase-1 x pipeline is gone
    and the first dots start ~8us in instead of ~16us.
  - With the transposes gone, the V projection moved into the head loop
    as PE filler.  PE is the pacing engine (~140us busy: QKV projections
    1.0 cycles/col in bf16, dots fp32r, attn@V bf16); ACT runs ~100us of
    softmax exp underneath it.  fp8/DoubleRow was measured (numpy) to
    push rel-err to 2.6e-2..6e-2 -- the cross-head variance statistics
    amplify product errors ~10x -- so everything stays bf16/fp32r.
  - attn@V runs in the [query, dim] orientation: exp tiles (bf16) are
    the PE stationary operand and V+ones (bf16) is the short (65-col)
    moving operand; the product lands directly in [q, d] layout.
  - attn_v lags dots/exp via a 30-buffer bf16 exp ring, and all PE
    filler (attn_v of earlier heads, K^T/Q^T chunk projections, V
    projection pieces) is emitted in small units INTERLEAVED between the
    dots tiles of each head: the dps double-buffer would otherwise stall
    the in-order PE queue behind a waiting dots matmul.
  - Tail: head 11 + normalize + mean/var + log-prob chain run PER QUERY
    TILE, pipelined across DVE (chains), ACT (squares, Ln, copies) and
    Pool, so the OH^T transposes + output projection (one continuous PE
    burst, bf16) start ~5us after the last exp instead of ~12us.  OH and
    W_out are bf16 (halves W_out DMA, 1.0 cyc/col transposes).
  - mean/var come from running partial sums (ACCS/ACCQ on Pool, head 11
    folded in at the tail), and the bias is folded into the output
    matmul as a rank-1 ones x bias product so y streams from fin tiles.
"""

import sys

if "/opt/trn_rl_repo" not in sys.path:
    sys.path.insert(0, "/opt/trn_rl_repo")

import ml_dtypes
import numpy as np

import concourse.bass as bass
import concourse.mybir as mybir
import concourse.tile as tile
from concourse import bacc
from concourse import bass_utils
from concourse.masks import make_identity

F32 = mybir.dt.float32
F32R = mybir.dt.float32r
BF16 = mybir.dt.bfloat16
AF = mybir.ActivationFunctionType
ALU = mybir.AluOpType
AX = mybir.AxisListType

B, N, E = 2, 2048, 768
H, DH = 12, 64
HD = H * DH            # 768
NQ = 512               # query rows per core
SCALE = DH ** -0.5
LOG2PI = float(np.log(2.0 * np.pi))
CONST = -0.5 * DH * LOG2PI   # -32*log(2*pi)

NE = E // 128          # 6 chunks of the embedding dim
NN = N // 128          # 16 chunks of the sequence
NQT = NQ // 128        # 4 query tiles
D1 = DH + 1            # head dim + sumexp column


def _emit(tc):
    nc = tc.nc
    xb = nc.dram_tensor("xb", [N, E], BF16, kind="ExternalInput")
    xb_ap = xb.ap()
    wqkv = nc.dram_tensor("wqkv", [E, 3 * HD], BF16, kind="ExternalInput")
    wqkv_ap = wqkv.ap()
    wout = nc.dram_tensor("wout", [HD, E], BF16, kind="ExternalInput").ap()
    bout_t = nc.dram_tensor("bout", [E], F32R, kind="ExternalInput")
    y = nc.dram_tensor("y", [NQ, E], F32, kind="ExternalOutput").ap()

    with tc.tile_pool(name="consts", bufs=1) as consts, \
         tc.tile_pool(name="big", bufs=1) as big, \
         tc.tile_pool(name="wop", bufs=1) as wop:
        # Load the one activation table that covers every function this
        # kernel uses (exp, ln, square, copy = set 6,
        # "natural_log_exp_and_others" in act_info.json).  Without this the
        # table-load pass picks "exp_and_others" first and a second 1283ns
        # load for Ln lands mid-chain in the latency-critical tail.
        nc.scalar.add_instruction(mybir.InstLoadActFuncSet(
            name=nc.scalar.bass.get_next_instruction_name(),
            act_func_set_id=6, ins=[], outs=[]))
        ident = consts.tile([128, 128], F32, name="ident", tag="ident")
        make_identity(nc, ident)
        identb = consts.tile([128, 128], BF16, name="identb", tag="identb")
        nc.vector.tensor_copy(identb, ident)
        ones_f = consts.tile([1, 128], F32, name="onesf", tag="onesf")
        nc.gpsimd.memset(ones_f, 1.0)
        ones_r = consts.tile([1, 128], F32R, name="ones", tag="ones")
        nc.vector.tensor_copy(ones_r, ones_f)

        # persistent SBUF tensors
        XT = [big.tile([128, N], BF16, name=f"xt{i}", tag=f"xt{i}")
              for i in range(NE)]
        VA = [big.tile([128, H, D1], BF16, name=f"va{j}", tag=f"va{j}")
              for j in range(NN)]
        PROD = big.tile([128, NQT, H, DH], BF16, name="prod", tag="prod")
        ACCS = big.tile([128, NQT, DH], F32, name="accs", tag="accs")
        ACCQ = big.tile([128, NQT, DH], F32, name="accq", tag="accq")
        MA = big.tile([128, NQT, DH], F32, name="ma", tag="ma")
        QA = big.tile([128, NQT, DH], F32, name="qa", tag="qa")
        OHQ = big.tile([128, NQT, H * DH], BF16, name="ohq", tag="ohq")
        bias = big.tile([1, E], F32R, name="bias", tag="bias")

        # ones column for the fused sumexp
        for va in VA:
            nc.gpsimd.memset(va[:, :, DH:D1], 1.0)

        with tc.tile_pool(name="jps", bufs=2, space="PSUM") as jps, \
             tc.tile_pool(name="wvp", bufs=1) as wvp, \
             tc.tile_pool(name="wqk", bufs=2) as wqk, \
             tc.tile_pool(name="ktp", bufs=2) as ktp, \
             tc.tile_pool(name="qtp", bufs=2) as qtp:

            # ---------------- K^T / Q^T projection helpers ----------------
            KT = {}
            QT = {}

            def load_wqk_pair(p):
                """W_q and W_k column chunks 2p,2p+1 as [128, 6, 256]
                tiles: paired loads give 512-byte DMA descriptors (half
                the per-descriptor latency of 256-byte ones) and half the
                HWDGE calls."""
                tq = wqk.tile([128, NE, 256], BF16, name="wq6", tag="wq6")
                tk = wqk.tile([128, NE, 256], BF16, name="wk6", tag="wk6")
                for t, col0 in ((tq, p * 256), (tk, HD + p * 256)):
                    nc.sync.dma_start(out=t, in_=bass.AP(
                        tensor=wqkv, offset=col0,
                        ap=[[3 * HD, 128], [128 * 3 * HD, NE], [1, 256]]))
                return tq, tk

            WVA = wvp.tile([128, NE, 512], BF16, name="wva", tag="wva")
            WVB = wvp.tile([128, NE, 256], BF16, name="wvb", tag="wvb")

            # -------- prologue DMA schedule (SP hwdge, priority order) ----
            # HWDGE issue is serialized at ~625ns/call, so DMAs are
            # consolidated (one strided call per W tensor) and ordered by
            # when the PE needs them: wqk0 + the query/first-key x^T spans
            # gate the first dots; later key spans gate the K^T pieces and
            # the V projection fillers.
            WQKP = {0: load_wqk_pair(0)}
            for r0 in (0, 512, 1024, 1536):
                for e in range(NE):
                    nc.sync.dma_start_transpose(
                        out=XT[e][:, r0:r0 + 512],
                        in_=xb_ap[r0:r0 + 512, e * 128:(e + 1) * 128])
            nc.sync.dma_start(out=WVA, in_=bass.AP(
                tensor=wqkv, offset=2 * HD,
                ap=[[3 * HD, 128], [128 * 3 * HD, NE], [1, 512]]))
            nc.sync.dma_start(out=WVB, in_=bass.AP(
                tensor=wqkv, offset=2 * HD + 512,
                ap=[[3 * HD, 128], [128 * 3 * HD, NE], [1, 256]]))

            # ---------------- V projection (filler units) -----------------
            def v_proj_a(nb):
                """V heads 0-7 for key block nb."""
                psA = jps.tile([128, 512], F32, name="vpa", tag="jp")
                for e in range(NE):
                    nc.tensor.matmul(
                        psA, XT[e][:, nb * 128:(nb + 1) * 128],
                        WVA[:, e, :],
                        start=(e == 0), stop=(e == NE - 1))
                nc.vector.tensor_copy(
                    VA[nb][:, 0:8, 0:DH],
                    psA.rearrange("p (h d) -> p h d", h=8))

            def v_proj_b(nb):
                """V heads 8-11 for key block nb."""
                psB = jps.tile([128, 512], F32, name="vpb", tag="jp")
                for e in range(NE):
                    nc.tensor.matmul(
                        psB[:, 0:256], XT[e][:, nb * 128:(nb + 1) * 128],
                        WVB[:, e, :],
                        start=(e == 0), stop=(e == NE - 1))
                nc.vector.tensor_copy(
                    VA[nb][:, 8:12, 0:DH],
                    psB[:, 0:256].rearrange("p (h d) -> p h d", h=4))

            def qk_proj_units(kc, tq, tk):
                """Closures: Q^T piece then 4 K^T pieces for chunk kc.
                tq/tk hold a kc-PAIR; co selects this kc's 128 columns."""
                co = (kc % 2) * 128
                kt = ktp.tile([128, N], F32R, name="kt", tag="kt")
                qt = qtp.tile([128, NQ], F32R, name="qt", tag="qt")
                KT[kc] = kt
                QT[kc] = qt

                def q_piece():
                    ps = jps.tile([128, 512], F32, name="qp", tag="jp")
                    for e in range(NE):
                        nc.tensor.matmul(ps, tq[:, e, co:co + 128],
                                         XT[e][:, 0:NQ],
                                         start=(e == 0), stop=(e == NE - 1))
                    nc.vector.tensor_copy(qt, ps)

                def k_piece(nb):
                    ps = jps.tile([128, 512], F32, name="kp", tag="jp")
                    for e in range(NE):
                        nc.tensor.matmul(
                            ps, tk[:, e, co:co + 128],
                            XT[e][:, nb * 512:(nb + 1) * 512],
                            start=(e == 0), stop=(e == NE - 1))
                    nc.vector.tensor_copy(kt[:, nb * 512:(nb + 1) * 512],
                                          ps)
                return [q_piece] + [
                    (lambda nb=nb: k_piece(nb)) for nb in range(4)]

            WO = []

            # ---------------- attention main loop -------------------------
            with tc.tile_pool(name="dps", bufs=2, space="PSUM") as dps, \
                 tc.tile_pool(name="pps", bufs=2, space="PSUM") as pps, \
                 tc.tile_pool(name="expp", bufs=30) as expp:

                # prologue PE: Q/K(0) projections gate the first dots
                u0 = qk_proj_units(0, *WQKP[0])
                u0[0]()
                u0[1]()
                PRE0 = {2: u0[2], 4: u0[3], 6: u0[4]}

                EXP = {}   # h -> list of 8 exp tiles
                PPS = {}   # h -> product psum tile

                def dots_exp(h, fillers=(), pre=None):
                    """dots+exp for head h with PE filler units emitted
                    between dots tiles (the dps double-buffer makes dots
                    exp-paced; interleaved filler keeps PE busy).  `pre`
                    maps tile index -> unit emitted before that tile (for
                    just-in-time K^T pieces of head 0)."""
                    fillers = list(fillers)
                    kc, pofs = h // 2, (h % 2) * DH
                    kt, qt = KT[kc], QT[kc]
                    qth = qt[pofs:pofs + DH, :]
                    exs = []
                    for jj in range(8):
                        if pre and jj in pre:
                            pre.pop(jj)()
                        dt_ = dps.tile([128, 2, NQ], F32, name="dots",
                                       tag="dots")
                        for k in range(2):
                            jb = jj * 2 + k
                            nc.tensor.matmul(
                                dt_[:, k, :],
                                kt[pofs:pofs + DH, jb * 128:(jb + 1) * 128],
                                qth, start=True, stop=True)
                        ex = expp.tile([128, 2, NQ], BF16, name="expd",
                                       tag="expd")
                        nc.scalar.activation(out=ex, in_=dt_, func=AF.Exp,
                                             scale=SCALE)
                        exs.append(ex)
                        if jj >= 1 and fillers:
                            fillers.pop(0)()
                    EXP[h] = exs
                    for f in fillers:
                        f()

                def attn_v_qt(h, qt_i):
                    """attn@V for head h, one query tile."""
                    exs = EXP[h]
                    if qt_i == 0:
                        PPS[h] = pps.tile([128, NQT, D1], F32, name="pp",
                                          tag="pp")
                    pp = PPS[h]
                    for jb in range(NN):
                        ex = exs[jb // 2]
                        st = ex[:, jb % 2, qt_i * 128:(qt_i + 1) * 128]
                        nc.tensor.matmul(
                            pp[:, qt_i, :], st, VA[jb][:, h, :],
                            start=(jb == 0), stop=(jb == NN - 1))
                    if qt_i == NQT - 1:
                        EXP.pop(h)

                def normalize(h):
                    """h <= 10: product write + running sums on Pool."""
                    acc_eng = nc.gpsimd
                    pp = PPS.pop(h)
                    rsh = big.tile([128, NQT], F32, name="rsh", tag="rsh",
                                   bufs=3)
                    nc.vector.reciprocal(rsh, pp[:, :, DH:D1])
                    pvh = bass.AP(tensor=PROD.tensor,
                                  offset=PROD.offset + h * DH,
                                  ap=[PROD.ap[0], [H * DH, NQT], [1, DH]])
                    rsh_bc = bass.AP(tensor=rsh.tensor, offset=rsh.offset,
                                     ap=[rsh.ap[0], [1, NQT], [0, DH]])
                    nc.vector.tensor_tensor(out=pvh, in0=pp[:, :, 0:DH],
                                            in1=rsh_bc, op=ALU.mult)
                    if h == 0:
                        acc_eng.tensor_copy(ACCS, pvh)
                        acc_eng.tensor_tensor(out=ACCQ, in0=pvh, in1=pvh,
                                              op=ALU.mult)
                    else:
                        sqh = big.tile([128, NQT, DH], F32, name="sqh",
                                       tag="sqh", bufs=2)
                        acc_eng.tensor_tensor(out=sqh, in0=pvh, in1=pvh,
                                              op=ALU.mult)
                        acc_eng.tensor_tensor(out=ACCS, in0=ACCS, in1=pvh,
                                              op=ALU.add)
                        acc_eng.tensor_tensor(out=ACCQ, in0=ACCQ, in1=sqh,
                                              op=ALU.add)

                done_av = 0

                def av_units(h):
                    """attn_v for head h as 4 filler units; normalize
                    rides with the last qt."""
                    us = [(lambda q=q: attn_v_qt(h, q))
                          for q in range(NQT - 1)]

                    def last():
                        attn_v_qt(h, NQT - 1)
                        normalize(h)
                    return us + [last]

                def drain_units(upto):
                    nonlocal done_av
                    us = []
                    while done_av < upto:
                        us += av_units(done_av)
                        done_av += 1
                    return us

                for h in range(H):
                    fillers = []
                    if h % 2 == 1 and h < H - 1:
                        if h in (1, 5):
                            # prefetch the next W_qk pair well ahead
                            WQKP[(h + 3) // 4] = load_wqk_pair((h + 3) // 4)
                        kc = h // 2 + 1
                        fillers += qk_proj_units(kc, *WQKP[kc // 2])
                    if h == 1:
                        fillers += [(lambda nb=nb: v_proj_a(nb))
                                    for nb in range(0, 8)]
                    if h == 2:
                        fillers = [(lambda nb=nb: v_proj_a(nb))
                                   for nb in range(8, 16)] \
                            + drain_units(2) + fillers
                    elif h >= 3:
                        upto = {8: 7, 9: 8, 10: 9, 11: 11}.get(h, h)
                        fillers = drain_units(upto) + fillers
                    if 3 <= h <= 7:
                        fillers += [(lambda k=k: v_proj_b(2 * (h - 3) + k))
                                    for k in range(2)]
                    elif h == 8:
                        fillers += [(lambda k=k: v_proj_b(k))
                                    for k in range(10, 13)]
                    elif h == 10:
                        fillers = [(lambda k=k: v_proj_b(k))
                                   for k in range(13, 16)] + fillers
                    if h == 8:
                        # W_out + bias loads, overlapped with late attention
                        nc.sync.dma_start(out=bias, in_=bass.AP(
                            tensor=bout_t, offset=0, ap=[[0, 1], [1, E]]))
                        wo_t = wop.tile([128, NE, E], BF16, name="wo",
                                        tag="wo")
                        nc.sync.dma_start(out=wo_t, in_=bass.AP(
                            tensor=wout.tensor, offset=0,
                            ap=[[E, 128], [128 * E, NE], [1, E]]))
                        WO.append(wo_t)
                    if h == H - 1:
                        def ma_qa():
                            nc.vector.tensor_scalar_mul(MA, ACCS, 1.0 / H)
                            nc.vector.tensor_scalar_mul(QA, ACCQ,
                                                        1.0 / (H - 1))
                        fillers.append(ma_qa)
                    dots_exp(h, fillers, pre=PRE0 if h == 0 else None)
                for u in drain_units(H - 1):
                    u()



                # ------- tail: head 11 + statistics -----------------------
                # attn_v(11) lands on PE back-to-back (a per-qt interleave
                # would WAR-serialize each start=True against normalize
                # reads through the PSUM zero region); then per-qt
                # normalize/mean/var/log-prob chains run on DVE (qt 0/2)
                # and Pool (qt 1/3) with squares + Ln on ACT, so the
                # output projection (emitted after this scope) starts on
                # qt 0 while later qts still compute.  Scratch transposes
                # reading the early stats tiles keep the tensor engine's
                # p-state ramp alive through the otherwise-idle window.
                for qt_i in range(NQT):
                    attn_v_qt(11, qt_i)
                warm = jps.tile([128, 128], F32, name="warm", tag="jp")

                def keep_warm(src, n):
                    out = warm[0:64, :]
                    for _ in range(n):
                        nc.tensor.transpose(out, src, ident)

                with tc.tile_pool(name="wkp", bufs=1) as wkp:
                    mean = wkp.tile([128, NQT, DH], F32, name="mean",
                                    tag="mean")
                    rvar = wkp.tile([128, NQT, DH], F32, name="rvar",
                                    tag="rvar")
                    cs = wkp.tile([128, NQT], F32, name="cs", tag="cs")
                    pp = PPS.pop(11)
                    for qt_i in range(NQT):
                        rshq = wkp.tile([128, 1], F32, name="rshq",
                                        tag="rshq", bufs=4)
                        nc.vector.reciprocal(rshq, pp[:, qt_i, DH:D1])
                        p11q = bass.AP(
                            tensor=PROD.tensor,
                            offset=PROD.offset + qt_i * H * DH + 11 * DH,
                            ap=[PROD.ap[0], [1, DH]])
                        rsh_bc = bass.AP(tensor=rshq.tensor,
                                         offset=rshq.offset,
                                         ap=[rshq.ap[0], [0, DH]])
                        nc.vector.tensor_tensor(out=p11q,
                                                in0=pp[:, qt_i, 0:DH],
                                                in1=rsh_bc, op=ALU.mult)
                        mq = mean[:, qt_i]
                        nc.vector.scalar_tensor_tensor(
                            out=mq, in0=p11q, scalar=1.0 / H,
                            in1=MA[:, qt_i], op0=ALU.mult, op1=ALU.add)
                        sqq = wkp.tile([128, DH], F32, name="sqq",
                                       tag="sqq", bufs=2)
                        nc.scalar.activation(out=sqq, in_=p11q,
                                             func=AF.Square)
                        qvq = wkp.tile([128, DH], F32, name="qvq",
                                       tag="qvq", bufs=2)
                        nc.vector.scalar_tensor_tensor(
                            out=qvq, in0=sqq, scalar=1.0 / (H - 1),
                            in1=QA[:, qt_i], op0=ALU.mult, op1=ALU.add)
                        m2q = wkp.tile([128, DH], F32, name="m2q",
                                       tag="m2q", bufs=2)
                        nc.vector.scalar_tensor_tensor(
                            out=m2q, in0=mq, scalar=H / (H - 1.0),
                            in1=mq, op0=ALU.mult, op1=ALU.mult)
                        varq = wkp.tile([128, DH], F32, name="varq",
                                        tag="varq", bufs=2)
                        nc.vector.tensor_tensor(out=varq, in0=qvq,
                                                in1=m2q, op=ALU.subtract)
                        rvq = rvar[:, qt_i]
                        nc.vector.reciprocal(rvq, varq)
                        lvq = wkp.tile([128, DH], F32, name="lvq",
                                       tag="lvq", bufs=2)
                        nc.scalar.activation(out=lvq, in_=varq, func=AF.Ln)
                        sq_ = wkp.tile([128, 1], F32, name="Sq", tag="Sq",
                                       bufs=2)
                        nc.vector.reduce_sum(sq_, lvq, axis=AX.X)
                        nc.vector.tensor_scalar(
                            out=cs[:, qt_i:qt_i + 1], in0=sq_, scalar1=-1.0,
                            scalar2=CONST, op0=ALU.mult, op1=ALU.add)
                        # log-prob chain over all heads for this qt
                        pvq = bass.AP(tensor=PROD.tensor,
                                      offset=PROD.offset + qt_i * H * DH,
                                      ap=[PROD.ap[0], [DH, H], [1, DH]])
                        diff = wkp.tile([128, H, DH], BF16, name="diff",
                                        tag="diff", bufs=2)
                        mean_bc = bass.AP(
                            tensor=mean.tensor,
                            offset=mean.offset + qt_i * DH,
                            ap=[mean.ap[0], [0, H], [1, DH]])
                        rvar_bc = bass.AP(
                            tensor=rvar.tensor,
                            offset=rvar.offset + qt_i * DH,
                            ap=[rvar.ap[0], [0, H], [1, DH]])
                        lp0 = wkp.tile([128, H], F32, name="lp0",
                                       tag="lp0", bufs=2)
                        lp = wkp.tile([128, H], F32, name="lp", tag="lp",
                                      bufs=2)
                        # qt 0 (which gates the whole output projection)
                        # splits heads 0-7 (DVE + ACT square) / 8-11 (Pool,
                        # in-engine square) so its first OH piece lands
                        # ~1.5us earlier; qt 2 runs on Pool, the rest DVE
                        if qt_i in (0, 3):
                            hsplits = [(0, 8, nc.vector, True),
                                       (8, H, nc.gpsimd, False)]
                        elif qt_i == 1:
                            hsplits = [(0, H, nc.gpsimd, True)]
                        else:
                            hsplits = [(0, H, nc.vector, True)]
                        for h0, h1, eng, act_sq in hsplits:
                            nh = h1 - h0
                            dfs = diff[:, h0:h1, :]
                            sub_bc = bass.AP(
                                tensor=mean.tensor,
                                offset=mean.offset + qt_i * DH,
                                ap=[mean.ap[0], [0, nh], [1, DH]])
                            pvs = bass.AP(
                                tensor=PROD.tensor,
                                offset=PROD.offset + qt_i * H * DH
                                + h0 * DH,
                                ap=[PROD.ap[0], [DH, nh], [1, DH]])
                            eng.tensor_tensor(out=dfs, in0=pvs,
                                              in1=sub_bc, op=ALU.subtract)
                            if act_sq:
                                nc.scalar.activation(out=dfs, in_=dfs,
                                                     func=AF.Square)
                            else:
                                eng.tensor_tensor(out=dfs, in0=dfs,
                                                  in1=dfs, op=ALU.mult)
                            rv_bc = bass.AP(
                                tensor=rvar.tensor,
                                offset=rvar.offset + qt_i * DH,
                                ap=[rvar.ap[0], [0, nh], [1, DH]])
                            eng.tensor_tensor(out=dfs, in0=dfs, in1=rv_bc,
                                              op=ALU.mult)
                            nc.vector.reduce_sum(lp0[:, h0:h1], dfs,
                                                 axis=AX.X)
                            nc.vector.tensor_scalar(
                                out=lp[:, h0:h1], in0=lp0[:, h0:h1],
                                scalar1=0.25,
                                scalar2=cs[:, qt_i:qt_i + 1],
                                op0=ALU.mult, op1=ALU.add)
                        # OH written in two pieces (heads 0-7, then 8-11)
                        # so the first four OH^T transposes (columns 0:512)
                        # start before the full tile is done
                        ohv_a = bass.AP(tensor=OHQ.tensor,
                                        offset=OHQ.offset + qt_i * H * DH,
                                        ap=[OHQ.ap[0], [DH, 8], [1, DH]])
                        ohv_b = bass.AP(
                            tensor=OHQ.tensor,
                            offset=OHQ.offset + qt_i * H * DH + 8 * DH,
                            ap=[OHQ.ap[0], [DH, 4], [1, DH]])
                        pvq_a = bass.AP(tensor=PROD.tensor,
                                        offset=PROD.offset + qt_i * H * DH,
                                        ap=[PROD.ap[0], [DH, 8], [1, DH]])
                        pvq_b = bass.AP(
                            tensor=PROD.tensor,
                            offset=PROD.offset + qt_i * H * DH + 8 * DH,
                            ap=[PROD.ap[0], [DH, 4], [1, DH]])
                        lp_a = bass.AP(tensor=lp.tensor, offset=lp.offset,
                                       ap=[lp.ap[0], [1, 8], [0, DH]])
                        lp_b = bass.AP(tensor=lp.tensor,
                                       offset=lp.offset + 8,
                                       ap=[lp.ap[0], [1, 4], [0, DH]])
                        eng_a = nc.gpsimd if qt_i == 1 else nc.vector
                        eng_b = nc.gpsimd if qt_i != 2 else nc.vector
                        eng_a.tensor_tensor(out=ohv_a, in0=pvq_a,
                                            in1=lp_a, op=ALU.mult)
                        eng_b.tensor_tensor(out=ohv_b, in0=pvq_b,
                                            in1=lp_b, op=ALU.mult)

        # ---------------- OH^T + output projection ------------------------
        # (attention PSUM pools are closed; tp2/fps reuse those banks)
        with tc.tile_pool(name="tp2", bufs=2, space="PSUM") as tp2p, \
             tc.tile_pool(name="fps", bufs=3, space="PSUM") as fps, \
             tc.tile_pool(name="ohtp", bufs=1) as ohtp, \
             tc.tile_pool(name="finp", bufs=2) as finp:
            OHT = ohtp.tile([128, NE, NQ], BF16, name="oht", tag="oht")
            for qt_i in range(NQT):
                oh_q = OHQ[:, qt_i]
                qs = slice(qt_i * 128, (qt_i + 1) * 128)
                # transposes collect in two PSUM tiles so the SBUF drain is
                # 2 batched copies instead of 6 (ACT access overhead)
                tp4 = tp2p.tile([128, 4, 128], BF16, name="t4", tag="t4")
                tp2 = tp2p.tile([128, 2, 128], BF16, name="t2", tag="t2")
                # chunks 0-3 (heads 0-7) transpose, drain, and start the
                # psA accumulation before the heads-8-11 OH piece is ready
                for c in range(4):
                    nc.tensor.transpose(
                        tp4[:, c, :], oh_q[:, c * 128:(c + 1) * 128],
                        identb)
                nc.scalar.copy(OHT[:, 0:4, qs], tp4)
                psA = fps.tile([128, 512], F32, name="fA", tag="f")
                psB = fps.tile([128, 256], F32, name="fB", tag="f")
                nc.tensor.matmul(psA, ones_r, bias[:, 0:512],
                                 start=True, stop=False)
                for c in range(4):
                    nc.tensor.matmul(psA, OHT[:, c, qs],
                                     WO[0][:, c, 0:512],
                                     start=False, stop=False)
                for c in (4, 5):
                    nc.tensor.transpose(
                        tp2[:, c - 4, :], oh_q[:, c * 128:(c + 1) * 128],
                        identb)
                nc.scalar.copy(OHT[:, 4:6, qs], tp2)
                for c in (4, 5):
                    nc.tensor.matmul(psA, OHT[:, c, qs],
                                     WO[0][:, c, 0:512],
                                     start=False, stop=(c == NE - 1))
                nc.tensor.matmul(psB, ones_r, bias[:, 512:768],
                                 start=True, stop=False)
                for c in range(NE):
                    nc.tensor.matmul(psB, OHT[:, c, qs],
                                     WO[0][:, c, 512:768],
                                     start=False, stop=(c == NE - 1))
                fin = finp.tile([128, E], F32, name="fin", tag="fin")
                nc.scalar.copy(fin[:, 0:512], psA)
                nc.sync.dma_start(out=y[qt_i * 128:(qt_i + 1) * 128, 0:512],
                                  in_=fin[:, 0:512])
                if qt_i == NQT - 1:
                    nc.vector.tensor_copy(fin[:, 512:768], psB)
                else:
                    nc.scalar.copy(fin[:, 512:768], psB)
                nc.sync.dma_start(out=y[qt_i * 128:(qt_i + 1) * 128,
                                        512:768],
                                  in_=fin[:, 512:768])


_NC_CACHE = {}


def _get_nc():
    if "nc" not in _NC_CACHE:
        nc = bacc.Bacc("TRN2", target_bir_lowering=False, debug=False,
                       num_devices=8)
        with tile.TileContext(nc) as tc:
            _emit(tc)
        nc.compile()
        _NC_CACHE["nc"] = nc
    return _NC_CACHE["nc"]


def kernel(x, w_qkv, w_out, b_out):
    x = np.ascontiguousarray(x, dtype=np.float32)
    w_qkv = np.ascontiguousarray(w_qkv, dtype=np.float32)
    w_out = np.ascontiguousarray(w_out, dtype=np.float32)
    b_out = np.ascontiguousarray(b_out, dtype=np.float32)
    assert x.shape == (B, N, E)

    nc = _get_nc()
    xb16 = x.astype(ml_dtypes.bfloat16)
    wq16 = np.ascontiguousarray(w_qkv.astype(ml_dtypes.bfloat16))
    wo16 = np.ascontiguousarray(w_out.astype(ml_dtypes.bfloat16))
    in_maps = []
    for c in range(8):
        beta, qoff = c // 4, (c % 4) * NQ
        xbc = np.ascontiguousarray(np.roll(xb16[beta], -qoff, axis=0))
        in_maps.append({"xb": xbc, "wqkv": wq16, "wout": wo16,
                        "bout": b_out})
    res = bass_utils.run_bass_kernel_spmd(nc, in_maps, core_ids=list(range(8)))
    out = np.empty((B, N, E), dtype=np.float32)
    for c in range(8):
        beta, qoff = c // 4, (c % 4) * NQ
        out[beta, qoff:qoff + NQ, :] = res.results[c]["y"]
    return out
